# revision 1
# baseline (speedup 1.0000x reference)
"""BitLinear (per-token int8 activation quant + ternary weight quant + matmul)
as a Bass/Tile kernel on 8 Trainium2 NeuronCores.

Strategy (data-parallel tokens / sharded weight prep):
  - x [4,2048,4096] -> [8192,4096]; each core gets a 1024-token slab.
  - weight [4096,4096]; core i gets out_feature rows [512i, 512(i+1)) for
    ternarization; the global mean(|W|) is assembled with a tiny AllReduce
    and ternarized slabs are shared with an AllGather (bf16, exact).
  - q = rint(x*s) (s = 127/max(|x|) per token) is exactly representable in
    bf16, tw in {-1,0,1} likewise, so the bf16 matmul with fp32 PSUM
    accumulation is EXACT integer arithmetic; dequant scales applied on the
    PSUM->SBUF copy.
  - Transposes (both operands need the contraction dim on partitions) are
    done by the DMA xbar transpose engine, keeping the PE free for matmuls.
"""
import numpy as np
from contextlib import ExitStack

N_CORES = 8
B, S, D_IN, D_OUT = 4, 2048, 4096, 4096
TOK = B * S                 # 8192
TOK_PC = TOK // N_CORES     # 1024 tokens per core
OF_PC = D_OUT // N_CORES    # 512 out-features ternarized per core
N_TOK_TILES = TOK_PC // 128  # 8
N_K = D_IN // 128            # 32 contraction tiles
OF_CHUNK = 512
N_SLAB = D_OUT // OF_CHUNK   # 8
EPS = 1e-5
MAGIC = float(np.float32(1.5 * 2 ** 23))   # fp32 round-to-nearest-even trick
MEAN_SCALE = float(np.float32(1.0 / (D_IN * D_OUT)))  # 2^-24, exact

_CACHE = {}


def _build_module():
    import concourse.bacc as bacc
    import concourse.tile as tile
    import concourse.mybir as mybir
    import concourse.bass_isa as bass_isa

    dt = mybir.dt
    AF = mybir.ActivationFunctionType
    AL = mybir.AluOpType
    AX = mybir.AxisListType

    nc = bacc.Bacc(
        "TRN2", target_bir_lowering=False, debug=False, num_devices=N_CORES
    )
    xs = nc.dram_tensor("xs", [TOK_PC, D_IN], dt.float32, kind="ExternalInput").ap()
    ws = nc.dram_tensor("ws", [OF_PC, D_IN], dt.float32, kind="ExternalInput").ap()
    out = nc.dram_tensor("out", [TOK_PC, D_OUT], dt.float32, kind="ExternalOutput").ap()

    wsum_d = nc.dram_tensor("wsum_d", [128, 1], dt.float32).ap()
    wsum_sh = nc.dram_tensor("wsum_sh", [128, 1], dt.float32, addr_space="Shared").ap()
    tw_d = nc.dram_tensor("tw_d", [OF_PC, D_IN], dt.bfloat16).ap()
    tw_full = nc.dram_tensor(
        "tw_full", [D_OUT, D_IN], dt.bfloat16, addr_space="Shared"
    ).ap()

    with tile.TileContext(nc) as tc, ExitStack() as ctx:
        stats = ctx.enter_context(tc.tile_pool(name="stats", bufs=1))
        qT_pool = ctx.enter_context(tc.tile_pool(name="qT", bufs=N_TOK_TILES))
        pp = ctx.enter_context(tc.tile_pool(name="pp", bufs=6, space="PSUM"))

        amc = stats.tile([128, N_TOK_TILES], dt.float32, tag="amc")
        s_all = stats.tile([128, N_TOK_TILES], dt.float32, tag="s_all")
        dq = stats.tile([128, N_TOK_TILES], dt.float32, tag="dq")
        wme = stats.tile([128, 1], dt.float32, tag="wme")
        swt = stats.tile([128, 1], dt.float32, tag="swt")
        wp = stats.tile([128, 4], dt.float32, tag="wp")
        wsum_sb = stats.tile([128, 1], dt.float32, tag="wsum_sb")
        gsb = stats.tile([128, 1], dt.float32, tag="gsb")
        gtot = stats.tile([128, 1], dt.float32, tag="gtot")

        qT_tiles = []
        with (
            tc.tile_pool(name="big", bufs=4) as big,
            tc.tile_pool(name="qp", bufs=2) as qp,
        ):
            # ---------- W slab partial |W| sums (overlaps x-quant) ----------
            with nc.named_scope("wsum"):
                for j in range(OF_PC // 128):
                    wt = big.tile([128, D_IN], dt.float32, tag="big")
                    nc.sync.dma_start(wt[:], ws[j * 128:(j + 1) * 128, :])
                    nc.vector.tensor_reduce(
                        wp[:, j:j + 1], wt[:], axis=AX.X, op=AL.add,
                        apply_absolute_value=True,
                    )
                nc.vector.tensor_reduce(
                    wsum_sb[:], wp[:], axis=AX.X, op=AL.add
                )
                nc.sync.dma_start(wsum_d[:], wsum_sb[:])
                nc.gpsimd.collective_compute(
                    "AllReduce", AL.add,
                    replica_groups=[list(range(N_CORES))],
                    ins=[wsum_d[:]], outs=[wsum_sh[:]],
                )
                nc.sync.dma_start(gsb[:], wsum_sh[:])
                nc.gpsimd.partition_all_reduce(
                    gtot[:], gsb[:], channels=128, reduce_op=bass_isa.ReduceOp.add
                )
                # mean_c = max(total * 2^-24, EPS); s_w = 1/mean_c (bit-exact)
                nc.vector.tensor_scalar(
                    wme[:], gtot[:], MEAN_SCALE, EPS, op0=AL.mult, op1=AL.max
                )
                nc.vector.reciprocal(swt[:], wme[:])

            # ---------- per-token activation quant + transpose ----------
            with nc.named_scope("xquant"):
                for t in range(N_TOK_TILES):
                    xt = big.tile([128, D_IN], dt.float32, tag="big")
                    nc.sync.dma_start(xt[:], xs[t * 128:(t + 1) * 128, :])
                    nc.vector.tensor_reduce(
                        amc[:, t:t + 1], xt[:], axis=AX.X, op=AL.max,
                        apply_absolute_value=True,
                    )
                    nc.vector.tensor_scalar(
                        amc[:, t:t + 1], amc[:, t:t + 1], EPS, None, op0=AL.max
                    )
                    nc.vector.reciprocal(s_all[:, t:t + 1], amc[:, t:t + 1])
                    nc.vector.tensor_scalar(
                        s_all[:, t:t + 1], s_all[:, t:t + 1], 127.0, None,
                        op0=AL.mult,
                    )
                    xsc = big.tile([128, D_IN], dt.float32, tag="big")
                    nc.scalar.activation(
                        xsc[:], xt[:], AF.Copy, scale=s_all[:, t:t + 1]
                    )
                    qb = qp.tile([128, D_IN], dt.bfloat16, tag="qb")
                    nc.vector.tensor_scalar(
                        qb[:], xsc[:], MAGIC, MAGIC, op0=AL.add, op1=AL.subtract
                    )
                    qT_t = qT_pool.tile([128, N_K, 128], dt.bfloat16, tag="qT")
                    nc.sync.dma_start(qT_t[:], qb[:], transpose=True)
                    qT_tiles.append(qT_t)
                    # dq = amax_c * mean_c / 127  (per-token dequant factor)
                    nc.vector.tensor_scalar(
                        dq[:, t:t + 1], amc[:, t:t + 1], wme[:, 0:1],
                        float(np.float32(1.0 / 127.0)), op0=AL.mult, op1=AL.mult,
                    )

            # ---------- ternarize own W slab, AllGather ----------
            with nc.named_scope("terniarize"):
                for j in range(OF_PC // 128):
                    wt = big.tile([128, D_IN], dt.float32, tag="big")
                    nc.sync.dma_start(wt[:], ws[j * 128:(j + 1) * 128, :])
                    wsc = big.tile([128, D_IN], dt.float32, tag="big")
                    nc.scalar.activation(
                        wsc[:], wt[:], AF.Copy, scale=swt[:, 0:1]
                    )
                    twb = qp.tile([128, D_IN], dt.bfloat16, tag="qb")
                    nc.vector.tensor_scalar(
                        twb[:], wsc[:], MAGIC, MAGIC, op0=AL.add, op1=AL.subtract
                    )
                    twc = qp.tile([128, D_IN], dt.bfloat16, tag="twc")
                    nc.vector.tensor_scalar(
                        twc[:], twb[:], 1.0, -1.0, op0=AL.min, op1=AL.max
                    )
                    nc.sync.dma_start(tw_d[j * 128:(j + 1) * 128, :], twc[:])
                nc.gpsimd.collective_compute(
                    "AllGather", AL.bypass,
                    replica_groups=[list(range(N_CORES))],
                    ins=[tw_d[:]], outs=[tw_full[:]],
                )

        # ---------- matmul + dequant ----------
        with (
            tc.tile_pool(name="twT", bufs=2) as twTp,
            tc.tile_pool(name="op", bufs=3) as op,
        ):
            with nc.named_scope("matmul"):
                for c in range(N_SLAB):
                    twT_c = twTp.tile([128, N_K, OF_CHUNK], dt.bfloat16, tag="twT")
                    nc.sync.dma_start(
                        twT_c[:],
                        tw_full[c * OF_CHUNK:(c + 1) * OF_CHUNK, :],
                        transpose=True,
                    )
                    for t in range(N_TOK_TILES):
                        ps = pp.tile([128, OF_CHUNK], dt.float32, tag="ps")
                        for k in range(N_K):
                            nc.tensor.matmul(
                                ps[:], qT_tiles[t][:, k, :], twT_c[:, k, :],
                                start=(k == 0), stop=(k == N_K - 1),
                            )
                        ot = op.tile([128, OF_CHUNK], dt.float32, tag="ot")
                        nc.scalar.mul(ot[:], ps[:], dq[:, t:t + 1])
                        nc.sync.dma_start(
                            out[t * 128:(t + 1) * 128,
                                c * OF_CHUNK:(c + 1) * OF_CHUNK],
                            ot[:],
                        )

    nc.compile()
    return nc


def _get_module():
    if "nc" not in _CACHE:
        _CACHE["nc"] = _build_module()
    return _CACHE["nc"]


def kernel(x: np.ndarray, weight: np.ndarray) -> np.ndarray:
    from concourse.bass_utils import run_bass_kernel_spmd

    x = np.asarray(x, dtype=np.float32)
    weight = np.asarray(weight, dtype=np.float32)
    x2 = np.ascontiguousarray(x.reshape(TOK, D_IN))
    w2 = np.ascontiguousarray(weight)

    in_maps = [
        {
            "xs": x2[i * TOK_PC:(i + 1) * TOK_PC],
            "ws": w2[i * OF_PC:(i + 1) * OF_PC],
        }
        for i in range(N_CORES)
    ]
    nc = _get_module()
    res = run_bass_kernel_spmd(nc, in_maps, list(range(N_CORES)))
    out = np.concatenate([res.results[i]["out"] for i in range(N_CORES)], axis=0)
    return out.reshape(B, S, D_OUT)
